# revision 1
# baseline (speedup 1.0000x reference)
"""DeepWalk community-pooling kernel for 8 trn2 NeuronCores.

Pipeline (per core, SPMD identical program, per-core data):
  host: sort extended rows (N + multi duplicates) by community, pad each
        community to a multiple of 8 rows, deal communities per size-class
        round-robin onto 32 (core, lane) slots so every core/lane has an
        identical class profile. Rows are laid out lane-interleaved in
        512-row chunks. Inputs are shipped pre-transposed (features on
        partitions).
  device:
    mmA   : ds^T [40,512] x Wdp -> psum_A = [h_even(40) | 0 | h_odd(40)]
    reluA : ACT relu+bias -> hx bf16 [104,512]
    mm_h  : hx x Wfeat(h-part)  -> psum_B (acc)
    mm_xf : xf^T [42,512] x Wfeat(x-part)+flag -> psum_B (acc)
            flag channel = -32768 for padding rows => y = relu(...) = 0
    reluB : DVE (psum + b_feat) max 0 -> Y bf16, lanes at parts {0,32,64,96}+20
    lvl1  : tensor_reduce sum/max over groups of 8 rows -> g1
    lvl2  : per size-class dense tensor_reduce over k groups -> g2 (sum, max)
    mean  : g2sum * recip(count)  (host-provided reciprocals)
    final : per lane block j: relu(W_out^T [mean;max] + b_out) -> out [16, C4]
  host: gather per-lane outputs back to the global community order.
"""

import os
import sys

import numpy as np

sys.path.insert(0, "/opt/trn_rl_repo")

import ml_dtypes  # noqa: E402

BF16 = ml_dtypes.bfloat16

N = 2_000_000
M = 500_000
C = 50_000
D_OUT = 16
N_CORES = 8
N_LANES = 4  # per core
FLAG_PAD = -32768.0
W_DMA = 8192
RB_ACT = 768  # relu_B free-split: ACT does [0:RB_ACT], DVE the rest  # F-columns per input DMA super-tile (8 supers)


# ----------------------------------------------------------------------------
# Host-side planning
# ----------------------------------------------------------------------------

def _plan(community, multi_community_index, multi_community_nodes):
    """Sort/pad/shard rows. Returns per-core row sources + static layout."""
    seg = np.concatenate([community, multi_community_index]).astype(np.int64)
    src = np.concatenate(
        [np.arange(N, dtype=np.int64), multi_community_nodes.astype(np.int64)]
    )

    counts = np.bincount(seg, minlength=C)
    kcls = np.maximum((counts + 7) // 8, 1).astype(np.int64)  # class = #groups
    assert kcls.max() <= 32, f"community too large: {counts.max()} rows"

    order = np.argsort(seg, kind="stable")
    src_sorted = src[order]
    # start offset of each community's run in src_sorted
    starts = np.zeros(C + 1, dtype=np.int64)
    np.cumsum(counts, out=starts[1:])

    # communities per class, dealt round-robin to 32 (core,lane) slots
    classes = np.unique(kcls)
    slot_comms = [[[] for _ in range(N_LANES)] for _ in range(N_CORES)]
    n32 = {}  # class k -> communities per slot
    for k in classes:
        comms = np.nonzero(kcls == k)[0]
        nk = len(comms)
        n32[int(k)] = (nk + 31) // 32
        for i, g in enumerate(comms):
            s = i % 32
            slot_comms[s // N_LANES][s % N_LANES].append(int(g))
    classes = [int(k) for k in classes]

    # per-lane group/community layout (identical across all cores/lanes)
    lane_groups = sum(n32[k] * k for k in classes)
    c4 = sum(n32[k] for k in classes)  # community slots per lane
    c4p = ((c4 + 511) // 512) * 512
    lane_rows = lane_groups * 8
    lane_len = ((lane_rows + 1023) // 1024) * 1024
    R = N_LANES * lane_len
    F = R // 2
    assert F % 1024 == 0

    # class offsets (group units and community-slot units)
    a_k, c_k, ga, ca = {}, {}, 0, 0
    for k in classes:
        a_k[k] = ga
        c_k[k] = ca
        ga += n32[k] * k
        ca += n32[k]

    # per (core,lane): row source indices (-1 = padding), per-slot counts
    core_data = []
    for ci in range(N_CORES):
        lane_src = np.full((N_LANES, lane_len), -1, dtype=np.int64)
        lane_flag = np.full((N_LANES, lane_len), FLAG_PAD, dtype=np.float32)
        slot_count = np.zeros((N_LANES, c4p), dtype=np.int64)
        slot_comm = np.full((N_LANES, c4p), -1, dtype=np.int64)
        for lj in range(N_LANES):
            comms = slot_comms[ci][lj]
            # group communities by class in class order; fakes implicit
            by_k = {k: [] for k in classes}
            for g in comms:
                by_k[int(kcls[g])].append(g)
            pos = 0
            for k in classes:
                lst = by_k[k]
                for i in range(n32[k]):
                    slot = c_k[k] + i
                    if i < len(lst):
                        g = lst[i]
                        cnt = int(counts[g])
                        s0 = starts[g]
                        lane_src[lj, pos : pos + cnt] = src_sorted[s0 : s0 + cnt]
                        lane_flag[lj, pos : pos + cnt] = 0.0
                        slot_count[lj, slot] = cnt
                        slot_comm[lj, slot] = g
                    pos += 8 * k
            assert pos == lane_rows
        core_data.append((lane_src, lane_flag, slot_count, slot_comm))

    layout = dict(
        classes=classes, n32=n32, a_k=a_k, c_k=c_k, R=R, F=F,
        c4=c4, c4p=c4p, lane_len=lane_len, lane_groups=lane_groups,
    )
    return core_data, layout


def _interleave_lanes(lane_mat, pair):
    """lane_mat [4, lane_len] -> 512-chunk interleaved stream of 2 lanes.

    pair=0 -> lanes (0, 2) (even chunks / chunk-c0), pair=1 -> lanes (1, 3).
    Returns [lane_len * 2] stream: chunks alternate lane pair[0], pair[1].
    """
    a = lane_mat[0 + pair].reshape(-1, 512)
    b = lane_mat[2 + pair].reshape(-1, 512)
    return np.stack([a, b], axis=1).reshape(-1)


def _build_core_inputs(core_dat, layout, x, dataset_x, params):
    """Build the DRAM images for one core."""
    lane_src, lane_flag, slot_count, _ = core_dat
    F = layout["F"]
    c4p = layout["c4p"]

    (W_demo, b_demo, W_purch, b_purch, W_feat, b_feat, W_out, b_out) = params

    ev_src = _interleave_lanes(lane_src, 0)
    od_src = _interleave_lanes(lane_src, 1)
    ev_flag = _interleave_lanes(lane_flag, 0)
    od_flag = _interleave_lanes(lane_flag, 1)

    ev_idx = np.maximum(ev_src, 0)
    od_idx = np.maximum(od_src, 0)

    ds = np.empty((40, F), dtype=BF16)
    ds[0:20] = dataset_x[ev_idx].T.astype(BF16)
    ds[20:40] = dataset_x[od_idx].T.astype(BF16)

    xf = np.empty((42, F), dtype=BF16)
    xf[0:20] = x[ev_idx].T.astype(BF16)
    xf[20] = ev_flag.astype(BF16)
    xf[21:41] = x[od_idx].T.astype(BF16)
    xf[41] = od_flag.astype(BF16)

    recip = np.ones((128, c4p), dtype=np.float32)
    for lj in range(N_LANES):
        r = 1.0 / np.maximum(slot_count[lj], 1).astype(np.float32)
        recip[32 * lj : 32 * lj + 20, :] = r[None, :]

    return dict(ds=ds, xf=xf, recip=recip)


def _build_shared_inputs(params):
    (W_demo, b_demo, W_purch, b_purch, W_feat, b_feat, W_out, b_out) = params

    # mmA stationary [128, 84]: ds_e rows 0-19 -> h_e cols 0-39,
    # ds_o rows 20-39 -> h_o cols 40-79, cols 80-83 zero pad
    wa = np.zeros((128, 84), dtype=BF16)
    wa[0:8, 0:20] = W_demo
    wa[8:20, 20:40] = W_purch
    wa[20:28, 40:60] = W_demo
    wa[28:40, 60:80] = W_purch

    # mmBIG stationary [128, 64]: hx rows -> y cols (e: 0-19, o: 32-51)
    wbig = np.zeros((128, 64), dtype=BF16)
    wbig[0:40, 0:20] = W_feat[0:40].astype(BF16)
    wbig[40:80, 32:52] = W_feat[0:40].astype(BF16)
    wbig[84:104, 0:20] = W_feat[40:60].astype(BF16)
    wbig[104, 0:20] = 1.0
    wbig[105:125, 32:52] = W_feat[40:60].astype(BF16)
    wbig[125, 32:52] = 1.0

    wout = np.zeros((128, 64), dtype=BF16)
    for lj in range(N_LANES):
        wout[32 * lj : 32 * lj + 20, 0:16] = W_out[0:20]
        wout[32 * lj : 32 * lj + 20, 32:48] = W_out[20:40]

    ba = np.zeros((128, 1), dtype=np.float32)
    ba[0:20, 0] = b_demo
    ba[20:40, 0] = b_purch
    ba[40:60, 0] = b_demo
    ba[60:80, 0] = b_purch

    bb = np.zeros((128, 1), dtype=np.float32)
    bo = np.zeros((128, 1), dtype=np.float32)
    for lj in range(N_LANES):
        bb[32 * lj : 32 * lj + 20, 0] = b_feat
        bo[32 * lj : 32 * lj + 16, 0] = b_out

    return dict(wa=wa, wbig=wbig, wout=wout, ba=ba, bb=bb, bo=bo)


# ----------------------------------------------------------------------------
# Device kernel
# ----------------------------------------------------------------------------

def _build_nc(layout):
    import concourse.bacc as bacc
    import concourse.mybir as mybir
    from concourse import tile

    f32 = mybir.dt.float32
    bf16 = mybir.dt.bfloat16

    F = layout["F"]
    c4p = layout["c4p"]
    n_supers = layout["R"] // 2048
    G1 = n_supers * 64
    classes = layout["classes"]
    n32 = layout["n32"]
    a_k = layout["a_k"]
    c_k = layout["c_k"]

    nc = bacc.Bacc("TRN2", target_bir_lowering=False, debug=False)

    dt_map = dict(ds=bf16, xf=bf16, recip=f32, wa=bf16, wbig=bf16, wout=bf16,
                  ba=f32, bb=f32, bo=f32)
    shapes = dict(ds=[40, F], xf=[42, F], recip=[128, c4p], wa=[128, 84],
                  wbig=[128, 64], wout=[128, 64], ba=[128, 1], bb=[128, 1],
                  bo=[128, 1])
    dram = {
        name: nc.declare_dram_parameter(name, shapes[name], dt_map[name], isOutput=False)
        for name in shapes
    }
    out_d = nc.declare_dram_parameter("out", [112, c4p], f32, isOutput=True)

    AX = mybir.AxisListType.X
    OP = mybir.AluOpType
    RELU = mybir.ActivationFunctionType.Relu

    with tile.TileContext(nc) as tc:
        with (
            tc.tile_pool(name="wpool", bufs=1) as wpool,
            tc.tile_pool(name="g", bufs=1) as gpool,
            tc.tile_pool(name="big", bufs=1) as bigp,
            tc.tile_pool(name="yp", bufs=3) as yp,
            tc.tile_pool(name="pa", bufs=2, space="PSUM") as pap,
            tc.tile_pool(name="pb", bufs=2, space="PSUM") as pbp,
            tc.tile_pool(name="outp", bufs=1) as outp,
        ):
            wa_t = wpool.tile([128, 84], bf16, tag="wa")
            wbig_t = wpool.tile([128, 64], bf16, tag="wbig")
            wout_t = wpool.tile([128, 64], bf16, tag="wout")
            ba_t = wpool.tile([128, 1], f32, tag="ba")
            bb_t = wpool.tile([128, 1], f32, tag="bb")
            bo_t = wpool.tile([128, 1], f32, tag="bo")
            recip_t = wpool.tile([128, c4p], f32, tag="recip")
            for name, t in [("wa", wa_t), ("wbig", wbig_t), ("wout", wout_t),
                            ("ba", ba_t), ("bb", bb_t), ("bo", bo_t),
                            ("recip", recip_t)]:
                nc.sync.dma_start(out=t[:], in_=dram[name][:])

            g1s = gpool.tile([128, G1], f32, tag="g1s")
            g1m = gpool.tile([128, G1], bf16, tag="g1m")
            g2s = gpool.tile([128, c4p], f32, tag="g2s")
            g2m = gpool.tile([128, c4p], bf16, tag="g2m")
            g2sb = gpool.tile([128, c4p], bf16, tag="g2sb")
            out_t = outp.tile([112, c4p], f32, tag="out")
            nc.gpsimd.memset(g2s[:, :], 0.0)
            nc.gpsimd.memset(g2m[:, :], 0.0)
            nc.gpsimd.memset(g2sb[:, :], 0.0)

            ds_t0 = bigp.tile([128, W_DMA], bf16, tag="ds0")
            ds_t1 = bigp.tile([128, W_DMA], bf16, tag="ds1")
            hx_t0 = bigp.tile([128, W_DMA], bf16, tag="hx0")
            hx_t1 = bigp.tile([128, W_DMA], bf16, tag="hx1")
            ds_tiles = [ds_t0, ds_t1]
            hx_tiles = [hx_t0, hx_t1]
            for t in ds_tiles:
                nc.gpsimd.memset(t[32:64, :], 0.0)
                nc.gpsimd.memset(t[64:128, :], 0.0)
            for t in hx_tiles:
                nc.gpsimd.memset(t[96:128, :], 0.0)

            lvl2_done = set()

            def _emit_lvl2(groups_ready):
                for k in classes:
                    if k in lvl2_done:
                        continue
                    nk = n32[k]
                    a = a_k[k]
                    if a + nk * k > groups_ready:
                        continue
                    c0 = c_k[k]
                    gv_s = g1s[0:116, a : a + nk * k].rearrange("p (n k) -> p n k", k=k)
                    gv_m = g1m[0:116, a : a + nk * k].rearrange("p (n k) -> p n k", k=k)
                    nc.vector.tensor_reduce(out=g2s[0:116, c0 : c0 + nk], in_=gv_s, axis=AX, op=OP.add)
                    nc.vector.tensor_reduce(out=g2m[0:116, c0 : c0 + nk], in_=gv_m, axis=AX, op=OP.max)
                    lvl2_done.add(k)

            for bi, blk0 in enumerate(range(0, F, W_DMA)):
                w_blk = min(W_DMA, F - blk0)
                ds_t = ds_tiles[bi % 2]
                hx_t = hx_tiles[bi % 2]
                nc.sync.dma_start(out=ds_t[0:40, :w_blk],
                                  in_=dram["ds"][:, blk0 : blk0 + w_blk])
                nc.sync.dma_start(out=hx_t[84:126, :w_blk],
                                  in_=dram["xf"][:, blk0 : blk0 + w_blk])

                for g_loc in range(w_blk // 2048):
                    g = (blk0 + g_loc * 2048) // 2048  # 2-super group index
                    pb = pbp.tile([128, 1024], f32, tag="pb")
                    for h in range(2):  # super within group
                        w0 = g_loc * 2048 + h * 1024
                        pa = pap.tile([128, 1024], f32, tag="pa")
                        for p in range(2):
                            nc.tensor.matmul(
                                pa[0:84, 512 * p : 512 * p + 512],
                                lhsT=wa_t[:, :],
                                rhs=ds_t[:, w0 + 512 * p : w0 + 512 * p + 512],
                                start=True, stop=True,
                            )
                        nc.scalar.activation(hx_t[0:84, w0 : w0 + 1024],
                                             pa[0:84, :], RELU, bias=ba_t[0:84, :])
                        for p in range(2):
                            nc.tensor.matmul(
                                pb[64 * p : 64 * p + 64, 512 * h : 512 * h + 512],
                                lhsT=wbig_t[:, :],
                                rhs=hx_t[:, w0 + 512 * p : w0 + 512 * p + 512],
                                start=True, stop=True,
                            )
                    y = yp.tile([116, 1024], bf16, tag="y")
                    nc.scalar.activation(y[0:116, 0:RB_ACT], pb[0:116, 0:RB_ACT],
                                         RELU, bias=bb_t[0:116, :])
                    nc.vector.tensor_scalar(
                        out=y[0:116, RB_ACT:1024], in0=pb[0:116, RB_ACT:1024],
                        scalar1=bb_t[0:116, :], scalar2=0.0,
                        op0=OP.add, op1=OP.max)
                    yv = y[0:116, :].rearrange("p (g k) -> p g k", k=8)
                    nc.vector.tensor_reduce(
                        out=g1s[0:116, 128 * g : 128 * g + 128], in_=yv, axis=AX, op=OP.add)
                    nc.vector.tensor_reduce(
                        out=g1m[0:116, 128 * g : 128 * g + 128], in_=yv, axis=AX, op=OP.max)
                    _emit_lvl2(128 * g + 128)

            _emit_lvl2(G1 * 2)

            nc.vector.tensor_mul(out=g2sb[0:116, :], in0=g2s[0:116, :], in1=recip_t[0:116, :])

            for cc in range(0, c4p, 512):
                po = pbp.tile([128, 1024], f32, tag="pb")
                for lj in range(N_LANES):
                    b0 = 32 * lj
                    nc.tensor.matmul(
                        po[b0 : b0 + 32, 0:512],
                        lhsT=wout_t[b0 : b0 + 20, 0:32],
                        rhs=g2sb[b0 : b0 + 20, cc : cc + 512],
                        start=True, stop=False, tile_position=(b0, b0),
                    )
                    nc.tensor.matmul(
                        po[b0 : b0 + 32, 0:512],
                        lhsT=wout_t[b0 : b0 + 20, 32:64],
                        rhs=g2m[b0 : b0 + 20, cc : cc + 512],
                        start=False, stop=True, tile_position=(b0, b0),
                    )
                nc.scalar.activation(
                    out_t[0:112, cc : cc + 512], po[0:112, 0:512], RELU, bias=bo_t[0:112, :])

            nc.sync.dma_start(out=out_d[:], in_=out_t[0:112, :])

    nc.compile()
    return nc


# ----------------------------------------------------------------------------
# Entry point
# ----------------------------------------------------------------------------

def kernel(x, dataset_x, community, multi_community_nodes, multi_community_index,
           W_demo, b_demo, W_purch, b_purch, W_feat, b_feat, W_out, b_out,
           _run_device=None):
    x = np.asarray(x, dtype=np.float32)
    dataset_x = np.asarray(dataset_x, dtype=np.float32)
    community = np.asarray(community)
    multi_community_nodes = np.asarray(multi_community_nodes)
    multi_community_index = np.asarray(multi_community_index)
    params = tuple(
        np.asarray(p, dtype=np.float32)
        for p in (W_demo, b_demo, W_purch, b_purch, W_feat, b_feat, W_out, b_out)
    )

    core_data, layout = _plan(community, multi_community_index, multi_community_nodes)
    shared = _build_shared_inputs(params)
    in_maps = []
    for ci in range(N_CORES):
        m = _build_core_inputs(core_data[ci], layout, x, dataset_x, params)
        m.update(shared)
        in_maps.append(m)

    if _run_device is None:
        from concourse.bass_utils import run_bass_kernel_spmd

        nc = _build_nc(layout)
        res = run_bass_kernel_spmd(nc, in_maps, list(range(N_CORES)))
        outs = [res.results[i]["out"] for i in range(N_CORES)]
    else:
        outs = _run_device(layout, in_maps)

    # gather per-lane outputs back to global community order
    OUT = np.zeros((C, D_OUT), dtype=np.float32)
    for ci in range(N_CORES):
        _, _, _, slot_comm = core_data[ci]
        oimg = np.asarray(outs[ci], dtype=np.float32)
        for lj in range(N_LANES):
            comms = slot_comm[lj]
            real = comms >= 0
            OUT[comms[real]] = oimg[32 * lj : 32 * lj + 16, : len(real)][:, real].T
    return OUT



# revision 7
# speedup vs baseline: 1.0346x; 1.0346x over previous
"""DeepWalk community-pooling kernel for 8 trn2 NeuronCores (v2).

Pipeline (per core, SPMD identical program, per-core data):
  host: sort extended rows (N + multi duplicates) by community, pad each
        community to a multiple of 8 rows, deal communities per size-class
        round-robin onto 48 (core, lane) slots (6 lanes/core) so every
        slot has an identical class profile.
  device, per 512-column "pb block" (512 stream indices x 6 lanes = 3072
  rows):
    mmA  : ds3^T 3-lane-packed [60,1024] x wa3 -> pa [120,1024] psum
    hx   : ACT relu+bias -> hx3 bf16 [120,1024]
    mm_h : wh^T x hx3 chunks -> pb[0:60] / pb[64:124] (accumulate)
    mm_xw: wxw^T x xf6 [126,512] 6-lane-packed -> pb (x-contribution +
           pad-flag), one matmul at 6-row/col density
    reluB: (pb + b_feat) relu -> y bf16 [124,512]  (ACT or DVE, balanced)
    sum  : DVE TT-tree radix-8 (2x bf16 mode) -> g1s
    max  : GPSIMD TT-tree radix-8 -> g1m
    lvl2 : per size-class tensor_reduce over k groups -> g2s (f32), g2m
  tail:  mean = g2s * recip (host-provided reciprocals), final GEMM
         relu(W_out^T [mean; max] + b_out) -> out [96, c6p]
  host: gather per-lane outputs back to the global community order.
"""

import sys

import numpy as np

sys.path.insert(0, "/opt/trn_rl_repo")

import ml_dtypes  # noqa: E402

BF16 = ml_dtypes.bfloat16

N = 2_000_000
M = 500_000
C = 50_000
D_OUT = 16
N_CORES = 8
N_LANES = 6  # per core
SLOTS = N_CORES * N_LANES
BLK = 512  # pb columns per block
FLAG_PAD = -32768.0
W3_DMA = 16384  # ds3 cols per input DMA tile (= 8192 stream idx)
LANE_OFF = [0, 20, 40, 64, 84, 104]  # partition offset of each lane block
RELUB_ACT_MOD = 2  # blocks with b % MOD == 1 do reluB on ACT, rest on DVE


# ----------------------------------------------------------------------------
# Host-side planning
# ----------------------------------------------------------------------------

def _plan(community, multi_community_index, multi_community_nodes):
    """Sort/pad/shard rows. Returns per-core row sources + static layout."""
    seg = np.concatenate([community, multi_community_index]).astype(np.int64)
    src = np.concatenate(
        [np.arange(N, dtype=np.int64), multi_community_nodes.astype(np.int64)]
    )

    counts = np.bincount(seg, minlength=C)
    kcls = np.maximum((counts + 7) // 8, 1).astype(np.int64)  # class = #groups
    assert kcls.max() <= 64, f"community too large: {counts.max()} rows"

    order = np.argsort(seg, kind="stable")
    src_sorted = src[order]
    starts = np.zeros(C + 1, dtype=np.int64)
    np.cumsum(counts, out=starts[1:])

    # communities per class, dealt round-robin to 48 (core,lane) slots
    classes = np.unique(kcls)
    slot_comms = [[[] for _ in range(N_LANES)] for _ in range(N_CORES)]
    n48 = {}  # class k -> communities per slot
    for k in classes:
        comms = np.nonzero(kcls == k)[0]
        n48[int(k)] = (len(comms) + SLOTS - 1) // SLOTS
        for i, g in enumerate(comms):
            s = i % SLOTS
            slot_comms[s // N_LANES][s % N_LANES].append(int(g))
    classes = [int(k) for k in classes]

    # per-lane group/community layout (identical across all cores/lanes)
    lane_groups = sum(n48[k] * k for k in classes)
    c6 = sum(n48[k] for k in classes)  # community slots per lane
    c6p = ((c6 + BLK - 1) // BLK) * BLK
    lane_rows = lane_groups * 8
    lane_len = ((lane_rows + BLK - 1) // BLK) * BLK

    # class offsets (group units and community-slot units)
    a_k, c_k, ga, ca = {}, {}, 0, 0
    for k in classes:
        a_k[k] = ga
        c_k[k] = ca
        ga += n48[k] * k
        ca += n48[k]

    # per (core,lane): row source indices (-1 = padding), per-slot counts
    core_data = []
    for ci in range(N_CORES):
        lane_src = np.full((N_LANES, lane_len), -1, dtype=np.int64)
        lane_flag = np.full((N_LANES, lane_len), FLAG_PAD, dtype=np.float32)
        slot_count = np.zeros((N_LANES, c6p), dtype=np.int64)
        slot_comm = np.full((N_LANES, c6p), -1, dtype=np.int64)
        for lj in range(N_LANES):
            comms = slot_comms[ci][lj]
            by_k = {k: [] for k in classes}
            for g in comms:
                by_k[int(kcls[g])].append(g)
            pos = 0
            for k in classes:
                lst = by_k[k]
                for i in range(n48[k]):
                    slot = c_k[k] + i
                    if i < len(lst):
                        g = lst[i]
                        cnt = int(counts[g])
                        s0 = starts[g]
                        lane_src[lj, pos : pos + cnt] = src_sorted[s0 : s0 + cnt]
                        lane_flag[lj, pos : pos + cnt] = 0.0
                        slot_count[lj, slot] = cnt
                        slot_comm[lj, slot] = g
                    pos += 8 * k
            assert pos == lane_rows
        core_data.append((lane_src, lane_flag, slot_count, slot_comm))

    layout = dict(
        classes=classes, n48=n48, a_k=a_k, c_k=c_k,
        c6=c6, c6p=c6p, lane_len=lane_len, lane_groups=lane_groups,
    )
    return core_data, layout


def _build_core_inputs(core_dat, layout, x, dataset_x):
    """Build the DRAM images for one core."""
    lane_src, lane_flag, slot_count, _ = core_dat
    lane_len = layout["lane_len"]
    c6p = layout["c6p"]
    nblk = lane_len // BLK
    F3 = 2 * lane_len
    F6 = lane_len

    idx = np.maximum(lane_src, 0)

    # ds3 [60, F3]: col 1024b+512t+j holds lanes {3t,3t+1,3t+2} at stream
    # index 512b+j; lane 3t+m occupies partitions 20m..20m+20.
    arr = dataset_x[idx].astype(BF16)              # [6, lane_len, 20]
    arrv = arr.reshape(2, 3, nblk, BLK, 20)        # [t, m, b, j, f]
    ds3 = np.ascontiguousarray(
        arrv.transpose(1, 4, 2, 0, 3).reshape(60, F3))

    # xf6 [126, F6]: col i holds all 6 lanes at stream index i;
    # lane l occupies partitions 21l..21l+20 (+ flag channel at 21l+20).
    xv = x[idx].astype(BF16)                       # [6, lane_len, 20]
    xf6 = np.empty((126, F6), dtype=BF16)
    for l in range(N_LANES):
        xf6[21 * l : 21 * l + 20] = xv[l].T
        xf6[21 * l + 20] = lane_flag[l].astype(BF16)

    recip = np.ones((124, c6p), dtype=np.float32)
    for l in range(N_LANES):
        r = 1.0 / np.maximum(slot_count[l], 1).astype(np.float32)
        off = LANE_OFF[l]
        recip[off : off + 20, :] = r[None, :]

    return dict(ds3=ds3, xf6=xf6, recip=recip)


def _build_shared_inputs(params):
    (W_demo, b_demo, W_purch, b_purch, W_feat, b_feat, W_out, b_out) = params

    # mmA stationary [60, 120]: 3 lanes; lane t ds feats at partitions
    # 20t..20t+20 -> h (demo|purch) at out cols 40t..40t+40
    wa3 = np.zeros((60, 120), dtype=BF16)
    for t in range(3):
        wa3[20 * t : 20 * t + 8, 40 * t : 40 * t + 20] = W_demo
        wa3[20 * t + 8 : 20 * t + 20, 40 * t + 20 : 40 * t + 40] = W_purch

    ba3 = np.zeros((120, 1), dtype=np.float32)
    for t in range(3):
        ba3[40 * t : 40 * t + 20, 0] = b_demo
        ba3[40 * t + 20 : 40 * t + 40, 0] = b_purch

    # mm_h stationary [120, 60]: lane t h-feats at 40t..40t+40 -> y cols
    # 20t..20t+20 (chunk A lanes 0-2 at pb[0:60], chunk B lanes 3-5 at
    # pb[64:124])
    wh = np.zeros((120, 60), dtype=BF16)
    for t in range(3):
        wh[40 * t : 40 * t + 40, 20 * t : 20 * t + 20] = W_feat[0:40]

    # mm_xw stationary [126, 124]: 6-lane-packed x -> x-part of y, plus the
    # pad flag channel -> -32768 on that lane's 20 y cols
    wxw = np.zeros((126, 124), dtype=BF16)
    for l in range(N_LANES):
        off = LANE_OFF[l]
        wxw[21 * l : 21 * l + 20, off : off + 20] = W_feat[40:60]
        wxw[21 * l + 20, off : off + 20] = 1.0

    bb6 = np.zeros((124, 1), dtype=np.float32)
    for l in range(N_LANES):
        off = LANE_OFF[l]
        bb6[off : off + 20, 0] = b_feat

    # final GEMM stationaries [124, 96]: lane l mean/max rows -> out cols
    # 16l..16l+16
    woutm = np.zeros((124, 96), dtype=BF16)
    woutx = np.zeros((124, 96), dtype=BF16)
    for l in range(N_LANES):
        off = LANE_OFF[l]
        woutm[off : off + 20, 16 * l : 16 * l + 16] = W_out[0:20]
        woutx[off : off + 20, 16 * l : 16 * l + 16] = W_out[20:40]

    bo6 = np.zeros((96, 1), dtype=np.float32)
    for l in range(N_LANES):
        bo6[16 * l : 16 * l + 16, 0] = b_out

    return dict(wa3=wa3, ba3=ba3, wh=wh, wxw=wxw, bb6=bb6,
                woutm=woutm, woutx=woutx, bo6=bo6)


# ----------------------------------------------------------------------------
# Device kernel
# ----------------------------------------------------------------------------

def _build_nc(layout):
    import concourse.bacc as bacc
    import concourse.mybir as mybir
    from concourse import tile

    f32 = mybir.dt.float32
    bf16 = mybir.dt.bfloat16

    lane_len = layout["lane_len"]
    c6p = layout["c6p"]
    nblk = lane_len // BLK
    F3 = 2 * lane_len
    F6 = lane_len
    G1 = nblk * 64  # lvl-1 group columns (64 per block)
    classes = layout["classes"]
    n48 = layout["n48"]
    a_k = layout["a_k"]
    c_k = layout["c_k"]

    nc = bacc.Bacc("TRN2", target_bir_lowering=False, debug=False)

    dt_map = dict(ds3=bf16, xf6=bf16, recip=f32, wa3=bf16, wh=bf16, wxw=bf16,
                  woutm=bf16, woutx=bf16, ba3=f32, bb6=f32, bo6=f32)
    shapes = dict(ds3=[60, F3], xf6=[126, F6], recip=[124, c6p],
                  wa3=[60, 120], wh=[120, 60], wxw=[126, 124],
                  woutm=[124, 96], woutx=[124, 96],
                  ba3=[120, 1], bb6=[124, 1], bo6=[96, 1])
    dram = {
        name: nc.declare_dram_parameter(name, shapes[name], dt_map[name],
                                        isOutput=False)
        for name in shapes
    }
    out_d = nc.declare_dram_parameter("out", [96, c6p], f32, isOutput=True)

    AX = mybir.AxisListType.X
    OP = mybir.AluOpType
    RELU = mybir.ActivationFunctionType.Relu

    with tile.TileContext(nc) as tc:
        with (
            tc.tile_pool(name="wpool", bufs=1) as wpool,
            tc.tile_pool(name="g", bufs=1) as gpool,
            tc.tile_pool(name="ds3p", bufs=2) as ds3p,
            tc.tile_pool(name="xf6p", bufs=2) as xf6p,
            tc.tile_pool(name="hxp", bufs=3) as hxp,
            tc.tile_pool(name="yp", bufs=3) as yp,
            tc.tile_pool(name="t1p", bufs=2) as t1p,
            tc.tile_pool(name="t2p", bufs=4) as t2p,
            tc.tile_pool(name="m1p", bufs=2) as m1p,
            tc.tile_pool(name="pa", bufs=2, space="PSUM") as pap,
            tc.tile_pool(name="pb", bufs=2, space="PSUM") as pbp,
            tc.tile_pool(name="outp", bufs=1) as outp,
        ):
            wa3_t = wpool.tile([60, 120], bf16, tag="wa3")
            wh_t = wpool.tile([120, 60], bf16, tag="wh")
            wxw_t = wpool.tile([126, 124], bf16, tag="wxw")
            woutm_t = wpool.tile([124, 96], bf16, tag="woutm")
            woutx_t = wpool.tile([124, 96], bf16, tag="woutx")
            ba3_t = wpool.tile([120, 1], f32, tag="ba3")
            bb6_t = wpool.tile([124, 1], f32, tag="bb6")
            bo6_t = wpool.tile([96, 1], f32, tag="bo6")
            recip_t = wpool.tile([124, c6p], f32, tag="recip")
            for name, t in [("wa3", wa3_t), ("wh", wh_t), ("wxw", wxw_t),
                            ("woutm", woutm_t), ("woutx", woutx_t),
                            ("ba3", ba3_t), ("bb6", bb6_t), ("bo6", bo6_t),
                            ("recip", recip_t)]:
                nc.sync.dma_start(out=t[:], in_=dram[name][:])

            g1s = gpool.tile([124, G1], bf16, tag="g1s")
            g1m = gpool.tile([124, G1], bf16, tag="g1m")
            g2s = gpool.tile([124, c6p], f32, tag="g2s")
            g2m = gpool.tile([124, c6p], bf16, tag="g2m")
            g2sb = gpool.tile([124, c6p], bf16, tag="g2sb")
            out_t = outp.tile([96, c6p], f32, tag="out")
            nc.gpsimd.memset(g2s[:, :], 0.0)
            nc.gpsimd.memset(g2m[:, :], 0.0)

            lvl2_done = set()

            def _emit_lvl2(groups_ready):
                for k in classes:
                    if k in lvl2_done:
                        continue
                    nk = n48[k]
                    a = a_k[k]
                    if a + nk * k > groups_ready:
                        continue
                    c0 = c_k[k]
                    gv_s = g1s[0:124, a : a + nk * k].rearrange(
                        "p (n k) -> p n k", k=k)
                    gv_m = g1m[0:124, a : a + nk * k].rearrange(
                        "p (n k) -> p n k", k=k)
                    nc.vector.tensor_reduce(
                        out=g2s[0:124, c0 : c0 + nk], in_=gv_s, axis=AX,
                        op=OP.add)
                    nc.vector.tensor_reduce(
                        out=g2m[0:124, c0 : c0 + nk], in_=gv_m, axis=AX,
                        op=OP.max)
                    lvl2_done.add(k)

            n_dma = (F3 + W3_DMA - 1) // W3_DMA
            for di in range(n_dma):
                o3 = di * W3_DMA
                w3 = min(W3_DMA, F3 - o3)
                o6, w6 = o3 // 2, w3 // 2
                ds3_t = ds3p.tile([60, W3_DMA], bf16, tag="ds3")
                xf6_t = xf6p.tile([126, W3_DMA // 2], bf16, tag="xf6")
                nc.sync.dma_start(out=ds3_t[:, :w3],
                                  in_=dram["ds3"][:, o3 : o3 + w3])
                nc.sync.dma_start(out=xf6_t[:, :w6],
                                  in_=dram["xf6"][:, o6 : o6 + w6])

                for bl in range(w6 // BLK):
                    b = o6 // BLK + bl  # global pb-block index
                    # --- stage 1: 3-lane-packed MLPs ---
                    pa = pap.tile([128, 1024], f32, tag="pa")
                    for t in range(2):
                        nc.tensor.matmul(
                            pa[0:120, 512 * t : 512 * t + 512],
                            lhsT=wa3_t[:, :],
                            rhs=ds3_t[0:60, 1024 * bl + 512 * t
                                      : 1024 * bl + 512 * t + 512],
                            start=True, stop=True,
                        )
                    hx = hxp.tile([120, 1024], bf16, tag="hx")
                    nc.scalar.activation(hx[:, :], pa[0:120, :], RELU,
                                         bias=ba3_t[:, :])
                    # --- stage 2: y pre-activation in pb ---
                    pb = pbp.tile([128, BLK], f32, tag="pb")
                    nc.tensor.matmul(
                        pb[0:124, :], lhsT=wxw_t[:, :],
                        rhs=xf6_t[0:126, BLK * bl : BLK * bl + BLK],
                        start=True, stop=False,
                    )
                    nc.tensor.matmul(
                        pb[0:60, :], lhsT=wh_t[:, 0:60],
                        rhs=hx[:, 0:512],
                        start=False, stop=True, skip_group_check=True,
                    )
                    nc.tensor.matmul(
                        pb[64:124, :], lhsT=wh_t[:, 0:60],
                        rhs=hx[:, 512:1024],
                        start=False, stop=True, skip_group_check=True,
                    )
                    # --- reluB ---
                    y = yp.tile([124, BLK], bf16, tag="y")
                    if b % RELUB_ACT_MOD == RELUB_ACT_MOD - 1:
                        nc.scalar.activation(y[:, :], pb[0:124, :], RELU,
                                             bias=bb6_t[:, :])
                    else:
                        nc.vector.tensor_scalar(
                            out=y[:, :], in0=pb[0:124, :],
                            scalar1=bb6_t[:, :], scalar2=0.0,
                            op0=OP.add, op1=OP.max)
                    # --- lvl-1 round 1 (radix 8 -> 4), per block ---
                    yv = y[:, :].rearrange("p (g k) -> p g k", k=8)
                    qoff = b % 4
                    if qoff == 0:
                        q0 = b
                        t1s = t1p.tile([124, 1024], bf16, tag="t1s")
                        t1m = m1p.tile([124, 1024], bf16, tag="t1m")
                    t1sv = t1s[:, 256 * qoff : 256 * qoff + 256].rearrange(
                        "p (g k) -> p g k", k=4)
                    t1mv = t1m[:, 256 * qoff : 256 * qoff + 256].rearrange(
                        "p (g k) -> p g k", k=4)
                    nc.vector.tensor_tensor(out=t1sv, in0=yv[:, :, 0:4],
                                            in1=yv[:, :, 4:8], op=OP.add)
                    nc.vector.tensor_tensor(out=t1mv, in0=yv[:, :, 0:4],
                                            in1=yv[:, :, 4:8], op=OP.max)
                    # --- rounds 2+3 batched over the quad ---
                    if qoff == 3 or b == nblk - 1:
                        nq = b - q0 + 1
                        for t1_, g1_, op_ in ((t1s, g1s, OP.add),
                                              (t1m, g1m, OP.max)):
                            tv = t1_[:, 0 : 256 * nq].rearrange(
                                "p (g k) -> p g k", k=4)
                            t2 = t2p.tile([124, 512], bf16, tag="t2")
                            t2v = t2[:, 0 : 128 * nq].rearrange(
                                "p (g k) -> p g k", k=2)
                            nc.vector.tensor_tensor(
                                out=t2v, in0=tv[:, :, 0:2],
                                in1=tv[:, :, 2:4], op=op_)
                            nc.vector.tensor_tensor(
                                out=g1_[0:124, 64 * q0 : 64 * (q0 + nq)],
                                in0=t2v[:, :, 0], in1=t2v[:, :, 1], op=op_)
                        _emit_lvl2(64 * (q0 + nq))

            nc.vector.tensor_mul(out=g2sb[:, :], in0=g2s[:, :],
                                 in1=recip_t[:, :])

            for cc in range(0, c6p, BLK):
                po = pbp.tile([128, BLK], f32, tag="pb")
                nc.tensor.matmul(
                    po[0:96, :], lhsT=woutm_t[:, :],
                    rhs=g2sb[0:124, cc : cc + BLK],
                    start=True, stop=False,
                )
                nc.tensor.matmul(
                    po[0:96, :], lhsT=woutx_t[:, :],
                    rhs=g2m[0:124, cc : cc + BLK],
                    start=False, stop=True,
                )
                nc.scalar.activation(out_t[0:96, cc : cc + BLK],
                                     po[0:96, :], RELU, bias=bo6_t[:, :])

            nc.sync.dma_start(out=out_d[:], in_=out_t[:, :])

    nc.compile()
    return nc


# ----------------------------------------------------------------------------
# Entry point
# ----------------------------------------------------------------------------

def _gather_output(core_data, outs):
    OUT = np.zeros((C, D_OUT), dtype=np.float32)
    for ci in range(N_CORES):
        _, _, _, slot_comm = core_data[ci]
        oimg = np.asarray(outs[ci], dtype=np.float32)
        for lj in range(N_LANES):
            comms = slot_comm[lj]
            real = comms >= 0
            OUT[comms[real]] = oimg[16 * lj : 16 * lj + 16, : len(real)][:, real].T
    return OUT


def kernel(x, dataset_x, community, multi_community_nodes, multi_community_index,
           W_demo, b_demo, W_purch, b_purch, W_feat, b_feat, W_out, b_out,
           _run_device=None):
    x = np.asarray(x, dtype=np.float32)
    dataset_x = np.asarray(dataset_x, dtype=np.float32)
    community = np.asarray(community)
    multi_community_nodes = np.asarray(multi_community_nodes)
    multi_community_index = np.asarray(multi_community_index)
    params = tuple(
        np.asarray(p, dtype=np.float32)
        for p in (W_demo, b_demo, W_purch, b_purch, W_feat, b_feat, W_out, b_out)
    )

    core_data, layout = _plan(community, multi_community_index,
                              multi_community_nodes)
    shared = _build_shared_inputs(params)
    in_maps = []
    for ci in range(N_CORES):
        m = _build_core_inputs(core_data[ci], layout, x, dataset_x)
        m.update(shared)
        in_maps.append(m)

    if _run_device is None:
        from concourse.bass_utils import run_bass_kernel_spmd

        nc = _build_nc(layout)
        res = run_bass_kernel_spmd(nc, in_maps, list(range(N_CORES)))
        outs = [res.results[i]["out"] for i in range(N_CORES)]
    else:
        outs = _run_device(layout, in_maps)

    return _gather_output(core_data, outs)


# revision 10
# speedup vs baseline: 1.0563x; 1.0210x over previous
"""DeepWalk community-pooling kernel for 8 trn2 NeuronCores (v2).

Pipeline (per core, SPMD identical program, per-core data):
  host: sort extended rows (N + multi duplicates) by community, pad each
        community to a multiple of 8 rows, deal communities per size-class
        round-robin onto 48 (core, lane) slots (6 lanes/core) so every
        slot has an identical class profile.
  device, per 512-column "pb block" (512 stream indices x 6 lanes = 3072
  rows):
    mmA  : ds3^T 3-lane-packed [60,1024] x wa3 -> pa [120,1024] psum
    hx   : ACT relu+bias -> hx3 bf16 [120,1024]
    mm_h : wh^T x hx3 chunks -> pb[0:60] / pb[64:124] (accumulate)
    mm_xw: wxw^T x xf6 [126,512] 6-lane-packed -> pb (x-contribution +
           pad-flag), one matmul at 6-row/col density
    reluB: (pb + b_feat) relu -> y bf16 [124,512]  (ACT or DVE, balanced)
    sum  : DVE TT-tree radix-8 (2x bf16 mode) -> g1s
    max  : GPSIMD TT-tree radix-8 -> g1m
    lvl2 : per size-class tensor_reduce over k groups -> g2s (f32), g2m
  tail:  mean = g2s * recip (host-provided reciprocals), final GEMM
         relu(W_out^T [mean; max] + b_out) -> out [96, c6p]
  host: gather per-lane outputs back to the global community order.
"""

import sys

import numpy as np

sys.path.insert(0, "/opt/trn_rl_repo")

import ml_dtypes  # noqa: E402

BF16 = ml_dtypes.bfloat16

N = 2_000_000
M = 500_000
C = 50_000
D_OUT = 16
N_CORES = 8
N_LANES = 6  # per core
SLOTS = N_CORES * N_LANES
BLK = 512  # pb columns per block
FLAG_PAD = -32768.0
W3_DMA = 16384  # ds3 cols per input DMA tile (= 8192 stream idx)
LANE_OFF = [0, 20, 40, 64, 84, 104]  # partition offset of each lane block
RELUB_ACT = frozenset({2, 5, 7})  # b % 8 in this set -> reluB on ACT
N_WARMUP = 12  # back-to-back warm-up matmuls to flip the PE HAM to 2.4 GHz


# ----------------------------------------------------------------------------
# Host-side planning
# ----------------------------------------------------------------------------

def _plan(community, multi_community_index, multi_community_nodes):
    """Sort/pad/shard rows. Returns per-core row sources + static layout."""
    seg = np.concatenate([community, multi_community_index]).astype(np.int64)
    src = np.concatenate(
        [np.arange(N, dtype=np.int64), multi_community_nodes.astype(np.int64)]
    )

    counts = np.bincount(seg, minlength=C)
    kcls = np.maximum((counts + 7) // 8, 1).astype(np.int64)  # class = #groups
    assert kcls.max() <= 64, f"community too large: {counts.max()} rows"

    order = np.argsort(seg, kind="stable")
    src_sorted = src[order]
    starts = np.zeros(C + 1, dtype=np.int64)
    np.cumsum(counts, out=starts[1:])

    # communities per class, dealt round-robin to 48 (core,lane) slots
    classes = np.unique(kcls)
    slot_comms = [[[] for _ in range(N_LANES)] for _ in range(N_CORES)]
    n48 = {}  # class k -> communities per slot
    for k in classes:
        comms = np.nonzero(kcls == k)[0]
        n48[int(k)] = (len(comms) + SLOTS - 1) // SLOTS
        for i, g in enumerate(comms):
            s = i % SLOTS
            slot_comms[s // N_LANES][s % N_LANES].append(int(g))
    classes = [int(k) for k in classes]

    # per-lane group/community layout (identical across all cores/lanes)
    lane_groups = sum(n48[k] * k for k in classes)
    c6 = sum(n48[k] for k in classes)  # community slots per lane
    c6p = ((c6 + BLK - 1) // BLK) * BLK
    lane_rows = lane_groups * 8
    lane_len = ((lane_rows + BLK - 1) // BLK) * BLK

    # class offsets (group units and community-slot units)
    a_k, c_k, ga, ca = {}, {}, 0, 0
    for k in classes:
        a_k[k] = ga
        c_k[k] = ca
        ga += n48[k] * k
        ca += n48[k]

    # per (core,lane): row source indices (-1 = padding), per-slot counts
    core_data = []
    for ci in range(N_CORES):
        lane_src = np.full((N_LANES, lane_len), -1, dtype=np.int64)
        lane_flag = np.full((N_LANES, lane_len), FLAG_PAD, dtype=np.float32)
        slot_count = np.zeros((N_LANES, c6p), dtype=np.int64)
        slot_comm = np.full((N_LANES, c6p), -1, dtype=np.int64)
        for lj in range(N_LANES):
            comms = slot_comms[ci][lj]
            by_k = {k: [] for k in classes}
            for g in comms:
                by_k[int(kcls[g])].append(g)
            pos = 0
            for k in classes:
                lst = by_k[k]
                for i in range(n48[k]):
                    slot = c_k[k] + i
                    if i < len(lst):
                        g = lst[i]
                        cnt = int(counts[g])
                        s0 = starts[g]
                        lane_src[lj, pos : pos + cnt] = src_sorted[s0 : s0 + cnt]
                        lane_flag[lj, pos : pos + cnt] = 0.0
                        slot_count[lj, slot] = cnt
                        slot_comm[lj, slot] = g
                    pos += 8 * k
            assert pos == lane_rows
        core_data.append((lane_src, lane_flag, slot_count, slot_comm))

    layout = dict(
        classes=classes, n48=n48, a_k=a_k, c_k=c_k,
        c6=c6, c6p=c6p, lane_len=lane_len, lane_groups=lane_groups,
    )
    return core_data, layout


def _build_core_inputs(core_dat, layout, x, dataset_x):
    """Build the DRAM images for one core."""
    lane_src, lane_flag, slot_count, _ = core_dat
    lane_len = layout["lane_len"]
    c6p = layout["c6p"]
    nblk = lane_len // BLK
    F3 = 2 * lane_len
    F6 = lane_len

    idx = np.maximum(lane_src, 0)

    # ds3 [60, F3]: col 1024b+512t+j holds lanes {3t,3t+1,3t+2} at stream
    # index 512b+j; lane 3t+m occupies partitions 20m..20m+20.
    arr = dataset_x[idx].astype(BF16)              # [6, lane_len, 20]
    arrv = arr.reshape(2, 3, nblk, BLK, 20)        # [t, m, b, j, f]
    ds3 = np.ascontiguousarray(
        arrv.transpose(1, 4, 2, 0, 3).reshape(60, F3))

    # xf6 [126, F6]: col i holds all 6 lanes at stream index i;
    # lane l occupies partitions 21l..21l+20 (+ flag channel at 21l+20).
    xv = x[idx].astype(BF16)                       # [6, lane_len, 20]
    xf6 = np.empty((126, F6), dtype=BF16)
    for l in range(N_LANES):
        xf6[21 * l : 21 * l + 20] = xv[l].T
        xf6[21 * l + 20] = lane_flag[l].astype(BF16)

    recip = np.ones((124, c6p), dtype=np.float32)
    for l in range(N_LANES):
        r = 1.0 / np.maximum(slot_count[l], 1).astype(np.float32)
        off = LANE_OFF[l]
        recip[off : off + 20, :] = r[None, :]

    return dict(ds3=ds3, xf6=xf6, recip=recip)


def _build_shared_inputs(params):
    (W_demo, b_demo, W_purch, b_purch, W_feat, b_feat, W_out, b_out) = params

    # mmA stationary [60, 120]: 3 lanes; lane t ds feats at partitions
    # 20t..20t+20 -> h (demo|purch) at out cols 40t..40t+40
    wa3 = np.zeros((60, 120), dtype=BF16)
    for t in range(3):
        wa3[20 * t : 20 * t + 8, 40 * t : 40 * t + 20] = W_demo
        wa3[20 * t + 8 : 20 * t + 20, 40 * t + 20 : 40 * t + 40] = W_purch

    ba3 = np.zeros((120, 1), dtype=np.float32)
    for t in range(3):
        ba3[40 * t : 40 * t + 20, 0] = b_demo
        ba3[40 * t + 20 : 40 * t + 40, 0] = b_purch

    # mm_h stationary [120, 60]: lane t h-feats at 40t..40t+40 -> y cols
    # 20t..20t+20 (chunk A lanes 0-2 at pb[0:60], chunk B lanes 3-5 at
    # pb[64:124])
    wh = np.zeros((120, 60), dtype=BF16)
    for t in range(3):
        wh[40 * t : 40 * t + 40, 20 * t : 20 * t + 20] = W_feat[0:40]

    # mm_xw stationary [126, 124]: 6-lane-packed x -> x-part of y, plus the
    # pad flag channel -> -32768 on that lane's 20 y cols
    wxw = np.zeros((126, 124), dtype=BF16)
    for l in range(N_LANES):
        off = LANE_OFF[l]
        wxw[21 * l : 21 * l + 20, off : off + 20] = W_feat[40:60]
        wxw[21 * l + 20, off : off + 20] = 1.0

    bb6 = np.zeros((124, 1), dtype=np.float32)
    for l in range(N_LANES):
        off = LANE_OFF[l]
        bb6[off : off + 20, 0] = b_feat

    # final GEMM stationaries [124, 96]: lane l mean/max rows -> out cols
    # 16l..16l+16
    woutm = np.zeros((124, 96), dtype=BF16)
    woutx = np.zeros((124, 96), dtype=BF16)
    for l in range(N_LANES):
        off = LANE_OFF[l]
        woutm[off : off + 20, 16 * l : 16 * l + 16] = W_out[0:20]
        woutx[off : off + 20, 16 * l : 16 * l + 16] = W_out[20:40]

    bo6 = np.zeros((96, 1), dtype=np.float32)
    for l in range(N_LANES):
        bo6[16 * l : 16 * l + 16, 0] = b_out

    return dict(wa3=wa3, ba3=ba3, wh=wh, wxw=wxw, bb6=bb6,
                woutm=woutm, woutx=woutx, bo6=bo6)


# ----------------------------------------------------------------------------
# Device kernel
# ----------------------------------------------------------------------------

def _build_nc(layout):
    import concourse.bacc as bacc
    import concourse.mybir as mybir
    from concourse import tile

    f32 = mybir.dt.float32
    bf16 = mybir.dt.bfloat16

    lane_len = layout["lane_len"]
    c6p = layout["c6p"]
    nblk = lane_len // BLK
    F3 = 2 * lane_len
    F6 = lane_len
    G1 = nblk * 64  # lvl-1 group columns (64 per block)
    classes = layout["classes"]
    n48 = layout["n48"]
    a_k = layout["a_k"]
    c_k = layout["c_k"]

    nc = bacc.Bacc("TRN2", target_bir_lowering=False, debug=False)

    dt_map = dict(ds3=bf16, xf6=bf16, recip=f32, wa3=bf16, wh=bf16, wxw=bf16,
                  woutm=bf16, woutx=bf16, ba3=f32, bb6=f32, bo6=f32)
    shapes = dict(ds3=[60, F3], xf6=[126, F6], recip=[124, c6p],
                  wa3=[60, 120], wh=[120, 60], wxw=[126, 124],
                  woutm=[124, 96], woutx=[124, 96],
                  ba3=[120, 1], bb6=[124, 1], bo6=[96, 1])
    dram = {
        name: nc.declare_dram_parameter(name, shapes[name], dt_map[name],
                                        isOutput=False)
        for name in shapes
    }
    out_d = nc.declare_dram_parameter("out", [96, c6p], f32, isOutput=True)

    AX = mybir.AxisListType.X
    OP = mybir.AluOpType
    RELU = mybir.ActivationFunctionType.Relu

    with tile.TileContext(nc) as tc:
        with (
            tc.tile_pool(name="wpool", bufs=1) as wpool,
            tc.tile_pool(name="g", bufs=1) as gpool,
            tc.tile_pool(name="ds3p", bufs=2) as ds3p,
            tc.tile_pool(name="xf6p", bufs=2) as xf6p,
            tc.tile_pool(name="hxp", bufs=3) as hxp,
            tc.tile_pool(name="yp", bufs=3) as yp,
            tc.tile_pool(name="t1p", bufs=2) as t1p,
            tc.tile_pool(name="t2p", bufs=4) as t2p,
            tc.tile_pool(name="m1p", bufs=2) as m1p,
            tc.tile_pool(name="pa", bufs=2, space="PSUM") as pap,
            tc.tile_pool(name="pb", bufs=2, space="PSUM") as pbp,
            tc.tile_pool(name="warmp", bufs=1, space="PSUM") as warmp,
            tc.tile_pool(name="outp", bufs=1) as outp,
        ):
            # PE HAM warm-up: ~12 dependency-free back-to-back matmuls give
            # the activity monitor one fully-busy 4096-cycle window, flipping
            # the PE clock gate from its default 1.2 GHz to 2.4 GHz for the
            # rest of the kernel (idle gaps < 3.4 us never re-throttle).
            wtmp = wpool.tile([128, 512], bf16, tag="wtmp")
            nc.gpsimd.memset(wtmp[:, :], 0.0)
            pw = warmp.tile([128, 512], f32, tag="warm")
            for _ in range(N_WARMUP):
                nc.tensor.matmul(pw[0:128, :], lhsT=wtmp[:, 0:128],
                                 rhs=wtmp[:, :], start=True, stop=True)
            wa3_t = wpool.tile([60, 120], bf16, tag="wa3")
            wh_t = wpool.tile([120, 60], bf16, tag="wh")
            wxw_t = wpool.tile([126, 124], bf16, tag="wxw")
            woutm_t = wpool.tile([124, 96], bf16, tag="woutm")
            woutx_t = wpool.tile([124, 96], bf16, tag="woutx")
            ba3_t = wpool.tile([120, 1], f32, tag="ba3")
            bb6_t = wpool.tile([124, 1], f32, tag="bb6")
            bo6_t = wpool.tile([96, 1], f32, tag="bo6")
            recip_t = wpool.tile([124, c6p], f32, tag="recip")
            for name, t in [("wa3", wa3_t), ("wh", wh_t), ("wxw", wxw_t),
                            ("woutm", woutm_t), ("woutx", woutx_t),
                            ("ba3", ba3_t), ("bb6", bb6_t), ("bo6", bo6_t),
                            ("recip", recip_t)]:
                nc.sync.dma_start(out=t[:], in_=dram[name][:])

            g1s = gpool.tile([124, G1], bf16, tag="g1s")
            g1m = gpool.tile([124, G1], bf16, tag="g1m")
            g2s = gpool.tile([124, c6p], f32, tag="g2s")
            g2m = gpool.tile([124, c6p], bf16, tag="g2m")
            g2sb = gpool.tile([124, c6p], bf16, tag="g2sb")
            out_t = outp.tile([96, c6p], f32, tag="out")
            nc.gpsimd.memset(g2s[:, :], 0.0)
            nc.gpsimd.memset(g2m[:, :], 0.0)

            lvl2_done = set()

            def _emit_lvl2(groups_ready):
                for k in classes:
                    if k in lvl2_done:
                        continue
                    nk = n48[k]
                    a = a_k[k]
                    if a + nk * k > groups_ready:
                        continue
                    c0 = c_k[k]
                    gv_s = g1s[0:124, a : a + nk * k].rearrange(
                        "p (n k) -> p n k", k=k)
                    gv_m = g1m[0:124, a : a + nk * k].rearrange(
                        "p (n k) -> p n k", k=k)
                    nc.vector.tensor_reduce(
                        out=g2s[0:124, c0 : c0 + nk], in_=gv_s, axis=AX,
                        op=OP.add)
                    nc.vector.tensor_reduce(
                        out=g2m[0:124, c0 : c0 + nk], in_=gv_m, axis=AX,
                        op=OP.max)
                    lvl2_done.add(k)

            n_dma = (F3 + W3_DMA - 1) // W3_DMA
            for di in range(n_dma):
                o3 = di * W3_DMA
                w3 = min(W3_DMA, F3 - o3)
                o6, w6 = o3 // 2, w3 // 2
                ds3_t = ds3p.tile([60, W3_DMA], bf16, tag="ds3")
                xf6_t = xf6p.tile([126, W3_DMA // 2], bf16, tag="xf6")
                nc.sync.dma_start(out=ds3_t[:, :w3],
                                  in_=dram["ds3"][:, o3 : o3 + w3])
                nc.sync.dma_start(out=xf6_t[:, :w6],
                                  in_=dram["xf6"][:, o6 : o6 + w6])

                for bl in range(w6 // BLK):
                    b = o6 // BLK + bl  # global pb-block index
                    # --- stage 1: 3-lane-packed MLPs ---
                    pa = pap.tile([128, 1024], f32, tag="pa")
                    for t in range(2):
                        nc.tensor.matmul(
                            pa[0:120, 512 * t : 512 * t + 512],
                            lhsT=wa3_t[:, :],
                            rhs=ds3_t[0:60, 1024 * bl + 512 * t
                                      : 1024 * bl + 512 * t + 512],
                            start=True, stop=True,
                        )
                    hx = hxp.tile([120, 1024], bf16, tag="hx")
                    nc.scalar.activation(hx[:, :], pa[0:120, :], RELU,
                                         bias=ba3_t[:, :])
                    # --- stage 2: y pre-activation in pb ---
                    pb = pbp.tile([128, BLK], f32, tag="pb")
                    nc.tensor.matmul(
                        pb[0:124, :], lhsT=wxw_t[:, :],
                        rhs=xf6_t[0:126, BLK * bl : BLK * bl + BLK],
                        start=True, stop=False,
                    )
                    nc.tensor.matmul(
                        pb[0:60, :], lhsT=wh_t[:, 0:60],
                        rhs=hx[:, 0:512],
                        start=False, stop=True, skip_group_check=True,
                    )
                    nc.tensor.matmul(
                        pb[64:124, :], lhsT=wh_t[:, 0:60],
                        rhs=hx[:, 512:1024],
                        start=False, stop=True, skip_group_check=True,
                    )
                    # --- reluB ---
                    y = yp.tile([124, BLK], bf16, tag="y")
                    if b % 8 in RELUB_ACT:
                        nc.scalar.activation(y[:, :], pb[0:124, :], RELU,
                                             bias=bb6_t[:, :])
                    else:
                        nc.vector.tensor_scalar(
                            out=y[:, :], in0=pb[0:124, :],
                            scalar1=bb6_t[:, :], scalar2=0.0,
                            op0=OP.add, op1=OP.max)
                    # --- lvl-1 round 1 (radix 8 -> 4), per block ---
                    yv = y[:, :].rearrange("p (g k) -> p g k", k=8)
                    qoff = b % 4
                    if qoff == 0:
                        q0 = b
                        t1s = t1p.tile([124, 1024], bf16, tag="t1s")
                        t1m = m1p.tile([124, 1024], bf16, tag="t1m")
                    t1sv = t1s[:, 256 * qoff : 256 * qoff + 256].rearrange(
                        "p (g k) -> p g k", k=4)
                    t1mv = t1m[:, 256 * qoff : 256 * qoff + 256].rearrange(
                        "p (g k) -> p g k", k=4)
                    nc.vector.tensor_tensor(out=t1sv, in0=yv[:, :, 0:4],
                                            in1=yv[:, :, 4:8], op=OP.add)
                    nc.vector.tensor_tensor(out=t1mv, in0=yv[:, :, 0:4],
                                            in1=yv[:, :, 4:8], op=OP.max)
                    # --- rounds 2+3 batched over the quad ---
                    if qoff == 3 or b == nblk - 1:
                        nq = b - q0 + 1
                        for t1_, g1_, op_ in ((t1s, g1s, OP.add),
                                              (t1m, g1m, OP.max)):
                            tv = t1_[:, 0 : 256 * nq].rearrange(
                                "p (g k) -> p g k", k=4)
                            t2 = t2p.tile([124, 512], bf16, tag="t2")
                            t2v = t2[:, 0 : 128 * nq].rearrange(
                                "p (g k) -> p g k", k=2)
                            nc.vector.tensor_tensor(
                                out=t2v, in0=tv[:, :, 0:2],
                                in1=tv[:, :, 2:4], op=op_)
                            nc.vector.tensor_tensor(
                                out=g1_[0:124, 64 * q0 : 64 * (q0 + nq)],
                                in0=t2v[:, :, 0], in1=t2v[:, :, 1], op=op_)
                        _emit_lvl2(64 * (q0 + nq))

            nc.vector.tensor_mul(out=g2sb[:, :], in0=g2s[:, :],
                                 in1=recip_t[:, :])

            for cc in range(0, c6p, BLK):
                po = pbp.tile([128, BLK], f32, tag="pb")
                nc.tensor.matmul(
                    po[0:96, :], lhsT=woutm_t[:, :],
                    rhs=g2sb[0:124, cc : cc + BLK],
                    start=True, stop=False,
                )
                nc.tensor.matmul(
                    po[0:96, :], lhsT=woutx_t[:, :],
                    rhs=g2m[0:124, cc : cc + BLK],
                    start=False, stop=True,
                )
                nc.scalar.activation(out_t[0:96, cc : cc + BLK],
                                     po[0:96, :], RELU, bias=bo6_t[:, :])

            nc.sync.dma_start(out=out_d[:], in_=out_t[:, :])

    nc.compile()
    return nc


# ----------------------------------------------------------------------------
# Entry point
# ----------------------------------------------------------------------------

def _gather_output(core_data, outs):
    OUT = np.zeros((C, D_OUT), dtype=np.float32)
    for ci in range(N_CORES):
        _, _, _, slot_comm = core_data[ci]
        oimg = np.asarray(outs[ci], dtype=np.float32)
        for lj in range(N_LANES):
            comms = slot_comm[lj]
            real = comms >= 0
            OUT[comms[real]] = oimg[16 * lj : 16 * lj + 16, : len(real)][:, real].T
    return OUT


def kernel(x, dataset_x, community, multi_community_nodes, multi_community_index,
           W_demo, b_demo, W_purch, b_purch, W_feat, b_feat, W_out, b_out,
           _run_device=None):
    x = np.asarray(x, dtype=np.float32)
    dataset_x = np.asarray(dataset_x, dtype=np.float32)
    community = np.asarray(community)
    multi_community_nodes = np.asarray(multi_community_nodes)
    multi_community_index = np.asarray(multi_community_index)
    params = tuple(
        np.asarray(p, dtype=np.float32)
        for p in (W_demo, b_demo, W_purch, b_purch, W_feat, b_feat, W_out, b_out)
    )

    core_data, layout = _plan(community, multi_community_index,
                              multi_community_nodes)
    shared = _build_shared_inputs(params)
    in_maps = []
    for ci in range(N_CORES):
        m = _build_core_inputs(core_data[ci], layout, x, dataset_x)
        m.update(shared)
        in_maps.append(m)

    if _run_device is None:
        from concourse.bass_utils import run_bass_kernel_spmd

        nc = _build_nc(layout)
        res = run_bass_kernel_spmd(nc, in_maps, list(range(N_CORES)))
        outs = [res.results[i]["out"] for i in range(N_CORES)]
    else:
        outs = _run_device(layout, in_maps)

    return _gather_output(core_data, outs)


# revision 20
# speedup vs baseline: 1.0992x; 1.0406x over previous
"""DeepWalk community-pooling kernel for 8 trn2 NeuronCores (v2).

Pipeline (per core, SPMD identical program, per-core data):
  host: sort extended rows (N + multi duplicates) by community, pad each
        community to a multiple of 8 rows, deal communities per size-class
        round-robin onto 48 (core, lane) slots (6 lanes/core) so every
        slot has an identical class profile.
  device, per 512-column "pb block" (512 stream indices x 6 lanes = 3072
  rows):
    mmA  : ds3^T 3-lane-packed [60,1024] x wa3 -> pa [120,1024] psum
    hx   : ACT relu+bias -> hx3 bf16 [120,1024]
    mm_h : wh^T x hx3 chunks -> pb[0:60] / pb[64:124] (accumulate)
    mm_xw: wxw^T x xf6 [126,512] 6-lane-packed -> pb (x-contribution +
           pad-flag), one matmul at 6-row/col density
    reluB: (pb + b_feat) relu -> y bf16 [124,512]  (ACT or DVE, balanced)
    sum  : DVE TT-tree radix-8 (2x bf16 mode) -> g1s
    max  : GPSIMD TT-tree radix-8 -> g1m
    lvl2 : per size-class tensor_reduce over k groups -> g2s (f32), g2m
  tail:  mean = g2s * recip (host-provided reciprocals), final GEMM
         relu(W_out^T [mean; max] + b_out) -> out [96, c6p]
  host: gather per-lane outputs back to the global community order.
"""

import sys

import numpy as np

sys.path.insert(0, "/opt/trn_rl_repo")

import ml_dtypes  # noqa: E402

BF16 = ml_dtypes.bfloat16
FP8 = ml_dtypes.float8_e4m3fn

N = 2_000_000
M = 500_000
C = 50_000
D_OUT = 16
N_CORES = 8
N_LANES = 6  # per core
SLOTS = N_CORES * N_LANES
BLK = 512  # pb columns per block
FLAG_PAD = -32768.0
W3_DMA = 8192  # ds3 cols per input DMA tile (= 4096 stream idx)
LANE_OFF = [0, 20, 40, 64, 84, 104]  # partition offset of each lane block
RELUB_ACT = frozenset({2, 5, 7})  # b % 8 in this set -> reluB on ACT
N_WARMUP = 12  # back-to-back warm-up matmuls to flip the PE HAM to 2.4 GHz


# ----------------------------------------------------------------------------
# Host-side planning
# ----------------------------------------------------------------------------

def _plan(community, multi_community_index, multi_community_nodes):
    """Sort/pad/shard rows. Returns per-core row sources + static layout."""
    seg = np.concatenate([community, multi_community_index]).astype(np.int64)
    src = np.concatenate(
        [np.arange(N, dtype=np.int64), multi_community_nodes.astype(np.int64)]
    )

    counts = np.bincount(seg, minlength=C)
    kcls = np.maximum((counts + 7) // 8, 1).astype(np.int64)  # class = #groups
    assert kcls.max() <= 64, f"community too large: {counts.max()} rows"

    order = np.argsort(seg, kind="stable")
    src_sorted = src[order]
    starts = np.zeros(C + 1, dtype=np.int64)
    np.cumsum(counts, out=starts[1:])

    # communities per class, dealt round-robin to 48 (core,lane) slots
    classes = np.unique(kcls)
    slot_comms = [[[] for _ in range(N_LANES)] for _ in range(N_CORES)]
    n48 = {}  # class k -> communities per slot
    for k in classes:
        comms = np.nonzero(kcls == k)[0]
        n48[int(k)] = (len(comms) + SLOTS - 1) // SLOTS
        for i, g in enumerate(comms):
            s = i % SLOTS
            slot_comms[s // N_LANES][s % N_LANES].append(int(g))
    classes = [int(k) for k in classes]

    # per-lane group/community layout (identical across all cores/lanes)
    lane_groups = sum(n48[k] * k for k in classes)
    c6 = sum(n48[k] for k in classes)  # community slots per lane
    c6p = ((c6 + BLK - 1) // BLK) * BLK
    lane_rows = lane_groups * 8
    lane_len = ((lane_rows + BLK - 1) // BLK) * BLK

    # class offsets (group units and community-slot units)
    a_k, c_k, ga, ca = {}, {}, 0, 0
    for k in classes:
        a_k[k] = ga
        c_k[k] = ca
        ga += n48[k] * k
        ca += n48[k]

    # per (core,lane): row source indices (-1 = padding), per-slot counts
    core_data = []
    for ci in range(N_CORES):
        lane_src = np.full((N_LANES, lane_len), -1, dtype=np.int64)
        lane_flag = np.full((N_LANES, lane_len), FLAG_PAD, dtype=np.float32)
        slot_count = np.zeros((N_LANES, c6p), dtype=np.int64)
        slot_comm = np.full((N_LANES, c6p), -1, dtype=np.int64)
        for lj in range(N_LANES):
            comms = slot_comms[ci][lj]
            by_k = {k: [] for k in classes}
            for g in comms:
                by_k[int(kcls[g])].append(g)
            pos = 0
            for k in classes:
                lst = by_k[k]
                for i in range(n48[k]):
                    slot = c_k[k] + i
                    if i < len(lst):
                        g = lst[i]
                        cnt = int(counts[g])
                        s0 = starts[g]
                        lane_src[lj, pos : pos + cnt] = src_sorted[s0 : s0 + cnt]
                        lane_flag[lj, pos : pos + cnt] = 0.0
                        slot_count[lj, slot] = cnt
                        slot_comm[lj, slot] = g
                    pos += 8 * k
            assert pos == lane_rows
        core_data.append((lane_src, lane_flag, slot_count, slot_comm))

    layout = dict(
        classes=classes, n48=n48, a_k=a_k, c_k=c_k,
        c6=c6, c6p=c6p, lane_len=lane_len, lane_groups=lane_groups,
    )
    return core_data, layout


def _build_core_inputs(core_dat, layout, x, dataset_x):
    """Build the DRAM images for one core."""
    lane_src, lane_flag, slot_count, _ = core_dat
    lane_len = layout["lane_len"]
    c6p = layout["c6p"]
    nblk = lane_len // BLK
    F3 = 2 * lane_len
    F6 = lane_len

    idx = np.maximum(lane_src, 0)

    # ds3 [60, F3]: col 1024b+512t+j holds lanes {3t,3t+1,3t+2} at stream
    # index 512b+j; lane 3t+m occupies partitions 20m..20m+20. fp8: the
    # demo/purch MLP path tolerates e4m3 (verified ~0.004 end-to-end).
    arr = dataset_x[idx].astype(FP8)               # [6, lane_len, 20]
    arrv = arr.reshape(2, 3, nblk, BLK, 20)        # [t, m, b, j, f]
    ds3 = np.ascontiguousarray(
        arrv.transpose(1, 4, 2, 0, 3).reshape(60, F3))

    # xf6 [126, F6]: col i holds all 6 lanes at stream index i;
    # lane l occupies partitions 21l..21l+20 (+ flag channel at 21l+20).
    xv = x[idx].astype(BF16)                       # [6, lane_len, 20]
    xf6 = np.empty((126, F6), dtype=BF16)
    for l in range(N_LANES):
        xf6[21 * l : 21 * l + 20] = xv[l].T
        xf6[21 * l + 20] = lane_flag[l].astype(BF16)

    recip = np.ones((124, c6p), dtype=np.float32)
    for l in range(N_LANES):
        r = 1.0 / np.maximum(slot_count[l], 1).astype(np.float32)
        off = LANE_OFF[l]
        recip[off : off + 20, :] = r[None, :]

    return dict(ds3=ds3, xf6=xf6, recip=recip)


def _build_shared_inputs(params):
    (W_demo, b_demo, W_purch, b_purch, W_feat, b_feat, W_out, b_out) = params

    # mmA stationary [60, 120]: 3 lanes; lane t ds feats at partitions
    # 20t..20t+20 -> h (demo|purch) at out cols 40t..40t+40
    wa3 = np.zeros((60, 120), dtype=FP8)
    for t in range(3):
        wa3[20 * t : 20 * t + 8, 40 * t : 40 * t + 20] = W_demo
        wa3[20 * t + 8 : 20 * t + 20, 40 * t + 20 : 40 * t + 40] = W_purch

    ba3 = np.zeros((120, 1), dtype=np.float32)
    for t in range(3):
        ba3[40 * t : 40 * t + 20, 0] = b_demo
        ba3[40 * t + 20 : 40 * t + 40, 0] = b_purch

    # mm_h stationary [120, 60]: lane t h-feats at 40t..40t+40 -> y cols
    # 20t..20t+20 (chunk A lanes 0-2 at pb[0:60], chunk B lanes 3-5 at
    # pb[64:124])
    wh = np.zeros((120, 60), dtype=BF16)
    for t in range(3):
        wh[40 * t : 40 * t + 40, 20 * t : 20 * t + 20] = W_feat[0:40]

    # mm_xw stationary [126, 124]: 6-lane-packed x -> x-part of y, plus the
    # pad flag channel -> -32768 on that lane's 20 y cols
    wxw = np.zeros((126, 124), dtype=BF16)
    for l in range(N_LANES):
        off = LANE_OFF[l]
        wxw[21 * l : 21 * l + 20, off : off + 20] = W_feat[40:60]
        wxw[21 * l + 20, off : off + 20] = 1.0

    bb6 = np.zeros((124, 1), dtype=np.float32)
    for l in range(N_LANES):
        off = LANE_OFF[l]
        bb6[off : off + 20, 0] = b_feat

    # final GEMM stationaries [124, 96]: lane l mean/max rows -> out cols
    # 16l..16l+16
    woutm = np.zeros((124, 96), dtype=BF16)
    woutx = np.zeros((124, 96), dtype=BF16)
    for l in range(N_LANES):
        off = LANE_OFF[l]
        woutm[off : off + 20, 16 * l : 16 * l + 16] = W_out[0:20]
        woutx[off : off + 20, 16 * l : 16 * l + 16] = W_out[20:40]

    bo6 = np.zeros((96, 1), dtype=np.float32)
    for l in range(N_LANES):
        bo6[16 * l : 16 * l + 16, 0] = b_out

    return dict(wa3=wa3, ba3=ba3, wh=wh, wxw=wxw, bb6=bb6,
                woutm=woutm, woutx=woutx, bo6=bo6)


# ----------------------------------------------------------------------------
# Device kernel
# ----------------------------------------------------------------------------

def _build_nc(layout):
    import concourse.bacc as bacc
    import concourse.mybir as mybir
    from concourse import tile

    f32 = mybir.dt.float32
    bf16 = mybir.dt.bfloat16
    f8 = mybir.dt.float8e4

    lane_len = layout["lane_len"]
    c6p = layout["c6p"]
    nblk = lane_len // BLK
    F3 = 2 * lane_len
    F6 = lane_len
    G1 = nblk * 64  # lvl-1 group columns (64 per block)
    classes = layout["classes"]
    n48 = layout["n48"]
    a_k = layout["a_k"]
    c_k = layout["c_k"]

    nc = bacc.Bacc("TRN2", target_bir_lowering=False, debug=False)

    dt_map = dict(ds3=f8, xf6=bf16, recip=f32, wa3=f8, wh=bf16, wxw=bf16,
                  woutm=bf16, woutx=bf16, ba3=f32, bb6=f32, bo6=f32)
    shapes = dict(ds3=[60, F3], xf6=[126, F6], recip=[124, c6p],
                  wa3=[60, 120], wh=[120, 60], wxw=[126, 124],
                  woutm=[124, 96], woutx=[124, 96],
                  ba3=[120, 1], bb6=[124, 1], bo6=[96, 1])
    dram = {
        name: nc.declare_dram_parameter(name, shapes[name], dt_map[name],
                                        isOutput=False)
        for name in shapes
    }
    out_d = nc.declare_dram_parameter("out", [96, c6p], f32, isOutput=True)

    AX = mybir.AxisListType.X
    OP = mybir.AluOpType
    RELU = mybir.ActivationFunctionType.Relu

    with tile.TileContext(nc) as tc:
        with (
            tc.tile_pool(name="wpool", bufs=1) as wpool,
            tc.tile_pool(name="g", bufs=1) as gpool,
            tc.tile_pool(name="ds3p", bufs=2) as ds3p,
            tc.tile_pool(name="xf6p", bufs=2) as xf6p,
            tc.tile_pool(name="hxp", bufs=3) as hxp,
            tc.tile_pool(name="yp", bufs=3) as yp,
            tc.tile_pool(name="t1p", bufs=2) as t1p,
            tc.tile_pool(name="t2p", bufs=4) as t2p,
            tc.tile_pool(name="m1p", bufs=2) as m1p,
            tc.tile_pool(name="pa", bufs=2, space="PSUM") as pap,
            tc.tile_pool(name="pb", bufs=2, space="PSUM") as pbp,
            tc.tile_pool(name="warmp", bufs=1, space="PSUM") as warmp,
            tc.tile_pool(name="outp", bufs=1) as outp,
        ):
            wa3_t = wpool.tile([60, 120], f8, tag="wa3")
            wh_t = wpool.tile([120, 60], bf16, tag="wh")
            wxw_t = wpool.tile([126, 124], bf16, tag="wxw")
            woutm_t = wpool.tile([124, 96], bf16, tag="woutm")
            woutx_t = wpool.tile([124, 96], bf16, tag="woutx")
            ba3_t = wpool.tile([120, 1], f32, tag="ba3")
            bb6_t = wpool.tile([124, 1], f32, tag="bb6")
            bo6_t = wpool.tile([96, 1], f32, tag="bo6")
            recip_t = wpool.tile([124, c6p], f32, tag="recip")
            for name, t in [("wa3", wa3_t), ("wh", wh_t), ("wxw", wxw_t),
                            ("woutm", woutm_t), ("woutx", woutx_t),
                            ("ba3", ba3_t), ("bb6", bb6_t), ("bo6", bo6_t),
                            ("recip", recip_t)]:
                nc.sync.dma_start(out=t[:], in_=dram[name][:])

            g1s = gpool.tile([124, G1], bf16, tag="g1s")
            g1m = gpool.tile([124, G1], bf16, tag="g1m")
            g2s = gpool.tile([124, c6p], f32, tag="g2s")
            g2m = gpool.tile([124, c6p], bf16, tag="g2m")
            g2sb = gpool.tile([124, c6p], bf16, tag="g2sb")
            out_t = outp.tile([96, c6p], f32, tag="out")
            nc.gpsimd.memset(g2s[:, :], 0.0)
            nc.gpsimd.memset(g2m[:, :], 0.0)

            lvl2_done = set()

            def _emit_lvl2(groups_ready):
                for k in classes:
                    if k in lvl2_done:
                        continue
                    nk = n48[k]
                    a = a_k[k]
                    if a + nk * k > groups_ready:
                        continue
                    c0 = c_k[k]
                    gv_s = g1s[0:124, a : a + nk * k].rearrange(
                        "p (n k) -> p n k", k=k)
                    gv_m = g1m[0:124, a : a + nk * k].rearrange(
                        "p (n k) -> p n k", k=k)
                    nc.vector.tensor_reduce(
                        out=g2s[0:124, c0 : c0 + nk], in_=gv_s, axis=AX,
                        op=OP.add)
                    nc.vector.tensor_reduce(
                        out=g2m[0:124, c0 : c0 + nk], in_=gv_m, axis=AX,
                        op=OP.max)
                    lvl2_done.add(k)

            n_dma = (F3 + W3_DMA - 1) // W3_DMA
            for di in range(n_dma):
                o3 = di * W3_DMA
                w3 = min(W3_DMA, F3 - o3)
                o6, w6 = o3 // 2, w3 // 2
                ds3_t = ds3p.tile([60, W3_DMA], f8, tag="ds3")
                xf6_t = xf6p.tile([126, W3_DMA // 2], bf16, tag="xf6")
                nc.sync.dma_start(out=ds3_t[:, :w3],
                                  in_=dram["ds3"][:, o3 : o3 + w3])
                nc.sync.dma_start(out=xf6_t[:, :w6],
                                  in_=dram["xf6"][:, o6 : o6 + w6])

                if di == 0:
                    # PE HAM warm-up, gated on the first input chunk: ~12
                    # back-to-back matmuls give the activity monitor one
                    # fully-busy 4096-cycle window, flipping the PE clock
                    # gate from its default 1.2 GHz to 2.4 GHz. Steady-state
                    # PE gaps stay well under the ~3.4 us idle window, so it
                    # never re-throttles.
                    pw = warmp.tile([128, 512], f32, tag="warm")
                    for _ in range(N_WARMUP):
                        nc.tensor.matmul(pw[0:120, :], lhsT=wa3_t[:, :],
                                         rhs=ds3_t[0:60, 0:512],
                                         start=True, stop=True)

                for bl in range(w6 // BLK):
                    b = o6 // BLK + bl  # global pb-block index
                    # --- stage 1: 3-lane-packed MLPs ---
                    pa = pap.tile([128, 1024], f32, tag="pa")
                    for t in range(2):
                        nc.tensor.matmul(
                            pa[0:120, 512 * t : 512 * t + 512],
                            lhsT=wa3_t[:, :],
                            rhs=ds3_t[0:60, 1024 * bl + 512 * t
                                      : 1024 * bl + 512 * t + 512],
                            start=True, stop=True,
                        )
                    hx = hxp.tile([120, 1024], bf16, tag="hx")
                    nc.scalar.activation(hx[:, :], pa[0:120, :], RELU,
                                         bias=ba3_t[:, :])
                    # --- stage 2: y pre-activation in pb ---
                    pb = pbp.tile([128, BLK], f32, tag="pb")
                    nc.tensor.matmul(
                        pb[0:124, :], lhsT=wxw_t[:, :],
                        rhs=xf6_t[0:126, BLK * bl : BLK * bl + BLK],
                        start=True, stop=False,
                    )
                    nc.tensor.matmul(
                        pb[0:60, :], lhsT=wh_t[:, 0:60],
                        rhs=hx[:, 0:512],
                        start=False, stop=True, skip_group_check=True,
                    )
                    nc.tensor.matmul(
                        pb[64:124, :], lhsT=wh_t[:, 0:60],
                        rhs=hx[:, 512:1024],
                        start=False, stop=True, skip_group_check=True,
                    )
                    # --- reluB ---
                    y = yp.tile([124, BLK], bf16, tag="y")
                    if b % 8 in RELUB_ACT:
                        nc.scalar.activation(y[:, :], pb[0:124, :], RELU,
                                             bias=bb6_t[:, :])
                    else:
                        nc.vector.tensor_scalar(
                            out=y[:, :], in0=pb[0:124, :],
                            scalar1=bb6_t[:, :], scalar2=0.0,
                            op0=OP.add, op1=OP.max)
                    # --- lvl-1 round 1 (radix 8 -> 4), per block ---
                    yv = y[:, :].rearrange("p (g k) -> p g k", k=8)
                    qoff = b % 4
                    if qoff == 0:
                        q0 = b
                        t1s = t1p.tile([124, 1024], bf16, tag="t1s")
                        t1m = m1p.tile([124, 1024], bf16, tag="t1m")
                    t1sv = t1s[:, 256 * qoff : 256 * qoff + 256].rearrange(
                        "p (g k) -> p g k", k=4)
                    t1mv = t1m[:, 256 * qoff : 256 * qoff + 256].rearrange(
                        "p (g k) -> p g k", k=4)
                    nc.vector.tensor_tensor(out=t1sv, in0=yv[:, :, 0:4],
                                            in1=yv[:, :, 4:8], op=OP.add)
                    nc.vector.tensor_tensor(out=t1mv, in0=yv[:, :, 0:4],
                                            in1=yv[:, :, 4:8], op=OP.max)
                    # --- rounds 2+3 batched over the quad ---
                    if qoff == 3 or b == nblk - 1:
                        nq = b - q0 + 1
                        for t1_, g1_, op_ in ((t1s, g1s, OP.add),
                                              (t1m, g1m, OP.max)):
                            tv = t1_[:, 0 : 256 * nq].rearrange(
                                "p (g k) -> p g k", k=4)
                            t2 = t2p.tile([124, 512], bf16, tag="t2")
                            t2v = t2[:, 0 : 128 * nq].rearrange(
                                "p (g k) -> p g k", k=2)
                            nc.vector.tensor_tensor(
                                out=t2v, in0=tv[:, :, 0:2],
                                in1=tv[:, :, 2:4], op=op_)
                            nc.vector.tensor_tensor(
                                out=g1_[0:124, 64 * q0 : 64 * (q0 + nq)],
                                in0=t2v[:, :, 0], in1=t2v[:, :, 1], op=op_)
                        _emit_lvl2(64 * (q0 + nq))

            nc.vector.tensor_mul(out=g2sb[:, :], in0=g2s[:, :],
                                 in1=recip_t[:, :])

            for cc in range(0, c6p, BLK):
                po = pbp.tile([128, BLK], f32, tag="pb")
                nc.tensor.matmul(
                    po[0:96, :], lhsT=woutm_t[:, :],
                    rhs=g2sb[0:124, cc : cc + BLK],
                    start=True, stop=False,
                )
                nc.tensor.matmul(
                    po[0:96, :], lhsT=woutx_t[:, :],
                    rhs=g2m[0:124, cc : cc + BLK],
                    start=False, stop=True,
                )
                nc.scalar.activation(out_t[0:96, cc : cc + BLK],
                                     po[0:96, :], RELU, bias=bo6_t[:, :])

            nc.sync.dma_start(out=out_d[:], in_=out_t[:, :])

    nc.compile()
    return nc


# ----------------------------------------------------------------------------
# Entry point
# ----------------------------------------------------------------------------

def _gather_output(core_data, outs):
    OUT = np.zeros((C, D_OUT), dtype=np.float32)
    for ci in range(N_CORES):
        _, _, _, slot_comm = core_data[ci]
        oimg = np.asarray(outs[ci], dtype=np.float32)
        for lj in range(N_LANES):
            comms = slot_comm[lj]
            real = comms >= 0
            OUT[comms[real]] = oimg[16 * lj : 16 * lj + 16, : len(real)][:, real].T
    return OUT


def kernel(x, dataset_x, community, multi_community_nodes, multi_community_index,
           W_demo, b_demo, W_purch, b_purch, W_feat, b_feat, W_out, b_out,
           _run_device=None):
    x = np.asarray(x, dtype=np.float32)
    dataset_x = np.asarray(dataset_x, dtype=np.float32)
    community = np.asarray(community)
    multi_community_nodes = np.asarray(multi_community_nodes)
    multi_community_index = np.asarray(multi_community_index)
    params = tuple(
        np.asarray(p, dtype=np.float32)
        for p in (W_demo, b_demo, W_purch, b_purch, W_feat, b_feat, W_out, b_out)
    )

    core_data, layout = _plan(community, multi_community_index,
                              multi_community_nodes)
    shared = _build_shared_inputs(params)
    in_maps = []
    for ci in range(N_CORES):
        m = _build_core_inputs(core_data[ci], layout, x, dataset_x)
        m.update(shared)
        in_maps.append(m)

    if _run_device is None:
        from concourse.bass_utils import run_bass_kernel_spmd

        nc = _build_nc(layout)
        res = run_bass_kernel_spmd(nc, in_maps, list(range(N_CORES)))
        outs = [res.results[i]["out"] for i in range(N_CORES)]
    else:
        outs = _run_device(layout, in_maps)

    return _gather_output(core_data, outs)
